# revision 6
# baseline (speedup 1.0000x reference)
"""GRUCell4RNMT fused Trainium2 kernel, data-parallel over 8 NeuronCores.

Reference computation (per batch row b):
    comb  = concat([x, h]) @ Wt.T            # [B, 2048]
    mu, var = moments over all 2048 comb features (joint LayerNorm)
    normed  = (comb - mu) * rsqrt(var+eps) * ln_w + ln_b
    ig, fg  = sigmoid(normed).split(2)
    hidden  = tanh(x @ Wi.T + bi + ig * (h @ Wh.T + bh))
    out     = (1 - fg) * hidden + fg * h

Strategy: shard batch 16384 -> 8 x 2048. On each core the three matmuls
share one pass over the contraction dim: host packs W_all[2048, 3072] =
[Wt.T | blockdiag(Wi.T, Wh.T)] so each stationary X^T k-tile streams 6
N=512 chunks (4 comb + 2 aux) into all 8 PSUM banks. Activations are
pre-transposed on host (X^T = [x;h]^T per core) so the device does no
transposes. Matmuls run in bf16 (1 cyc/row on PE); accumulation is fp32
in PSUM; LN statistics via bn_stats/bn_aggr; epilogue on DVE/ACT.
"""

import numpy as np
import ml_dtypes

from concourse import bass, mybir, tile
from concourse.bass_utils import run_bass_kernel_spmd
from concourse.vector_clock import ScopedClock

BF16 = ml_dtypes.bfloat16
F32 = mybir.dt.float32
BF = mybir.dt.bfloat16
AF = mybir.ActivationFunctionType
ALU = mybir.AluOpType

N_CORES = 8
B = 16384
O = 1024
BL = B // N_CORES          # 2048 rows per core
N_BT = BL // 128           # 16 batch tiles per core
N_K = 16                   # contraction tiles (8 from x, 8 from h)
LN_EPS = 1e-6


class _TC(tile.TileContext):
    """TileContext whose kernel-tail drain honors the 1-wait-per-
    instruction ISA cap: extra waits move onto dedicated drains."""

    def _drain_and_barrier(self, tick_clock, wait_clock):
        drain_inst = self.nc.sync.drain()
        wait_clock.add_sem_waits(
            drain_inst.ins, ScopedClock({None: tick_clock.global_clock})
        )
        si = drain_inst.ins.sync_info
        if si is not None and si.on_wait and len(si.on_wait) > 1:
            waits = list(si.on_wait)
            SI = type(si)
            si.on_wait = [waits[0]]
            for w in waits[1:]:
                extra = self.nc.sync.drain()
                extra.ins.sync_info = SI(on_wait=[w], on_update=[])
        self.nc.all_engine_barrier()
        assert self.sems is not None
        popped = self.nc._tile_sem_poison_stack.pop()
        assert popped is self._sem_poison
        self.nc.clear_and_free_semaphores(list(self.sems.allocated().values()))
        self.nc.all_engine_barrier()


def _split_multi_waits(nc):
    """This walrus build accepts 1 sync wait per instruction (2 on
    EventSemaphore). Tile's scheduler can emit more; move the extras
    onto EventSemaphore carriers inserted just before the offender on
    the same engine (identical blocking semantics)."""
    for fn in nc.m.functions:
        for blk in fn.blocks:
            il = blk.instructions
            i = 0
            while i < len(il):
                inst = il[i]
                si = inst.sync_info
                cap = 2 if isinstance(inst, mybir.InstEventSemaphore) else 1
                if si is not None and si.on_wait and len(si.on_wait) > cap:
                    waits = list(si.on_wait)
                    SI = type(si)
                    si.on_wait = waits[:cap]
                    extra = waits[cap:]
                    pos = i
                    while extra:
                        chunk, extra = extra[:2], extra[2:]
                        ev = mybir.InstEventSemaphore(
                            name=nc.get_next_instruction_name(), ins=[], outs=[]
                        )
                        ev.engine = inst.engine
                        ev.sync_info = SI(on_wait=chunk, on_update=[])
                        nc.register_instruction(ev, overwrite=True)
                        il.insert(pos, ev)
                        pos += 1
                        i += 1
                i += 1


def build_program(n_bt=N_BT):
    nc = bass.Bass()
    bl = n_bt * 128

    xt = nc.declare_dram_parameter("xt", [2048, bl], BF, isOutput=False)
    st = nc.declare_dram_parameter("st", [bl, O], F32, isOutput=False)
    w = nc.declare_dram_parameter("w", [2048, 3072], BF, isOutput=False)
    lnw = nc.declare_dram_parameter("lnw", [128, 2048], BF, isOutput=False)
    lnb = nc.declare_dram_parameter("lnb", [128, 2048], BF, isOutput=False)
    bib = nc.declare_dram_parameter("bib", [128, O], BF, isOutput=False)
    bhb = nc.declare_dram_parameter("bhb", [128, O], BF, isOutput=False)
    out = nc.declare_dram_parameter("out", [bl, O], F32, isOutput=True)

    # DRAM view of X^T tiled for per-batch-tile loads: [p, k, b*128+j]
    xt_r = xt[:].rearrange("(k p) n -> p k n", p=128)

    with _TC(nc) as tc:
        with (
            tc.tile_pool(name="wp", bufs=1) as wp,
            tc.tile_pool(name="cp", bufs=1) as cp,
            tc.tile_pool(name="xp", bufs=3) as xp,
            tc.tile_pool(name="sp", bufs=2) as sp,
            tc.tile_pool(name="ep", bufs=2) as ep,
            tc.tile_pool(name="pc_p", bufs=1, space="PSUM") as pc_p,
            tc.tile_pool(name="pa_p", bufs=1, space="PSUM") as pa_p,
        ):
            wt = []
            for k in range(N_K):
                wk = wp.tile([128, 3072], BF, tag=f"w{k}")
                nc.sync.dma_start(wk[:], w[k * 128:(k + 1) * 128, :])
                wt.append(wk)

            lnw_t = cp.tile([128, 2048], BF, tag="lnw")
            nc.sync.dma_start(lnw_t[:], lnw[:])
            lnb_t = cp.tile([128, 2048], BF, tag="lnb")
            nc.sync.dma_start(lnb_t[:], lnb[:])
            bib_t = cp.tile([128, O], BF, tag="bib")
            nc.sync.dma_start(bib_t[:], bib[:])
            bhb_t = cp.tile([128, O], BF, tag="bhb")
            nc.sync.dma_start(bhb_t[:], bhb[:])
            eps_t = cp.tile([128, 1], F32, tag="eps")
            nc.vector.memset(eps_t[:], LN_EPS)

            for bt in range(n_bt):
                xt_t = xp.tile([128, N_K, 128], BF, tag="xt")
                nc.sync.dma_start(
                    xt_t[:], xt_r[:, :, bt * 128:(bt + 1) * 128]
                )
                st_t = ep.tile([128, O], F32, tag="st")
                nc.sync.dma_start(st_t[:], st[bt * 128:(bt + 1) * 128, :])

                pc = pc_p.tile([128, 2048], F32, tag="pc")
                xwi = pa_p.tile([128, O], F32, tag="xwi")
                hwh = pa_p.tile([128, O], F32, tag="hwh")

                for k in range(N_K):
                    lhsT = xt_t[:, k, :]
                    wk = wt[k]
                    # aux chunks first: frees those banks earliest
                    for n in (4, 5):
                        dst = xwi if k < 8 else hwh
                        nc.tensor.matmul(
                            dst[:, (n - 4) * 512:(n - 3) * 512],
                            lhsT,
                            wk[:, n * 512:(n + 1) * 512],
                            start=(k % 8 == 0),
                            stop=(k % 8 == 7),
                        )
                    for n in range(4):
                        nc.tensor.matmul(
                            pc[:, n * 512:(n + 1) * 512],
                            lhsT,
                            wk[:, n * 512:(n + 1) * 512],
                            start=(k == 0),
                            stop=(k == N_K - 1),
                        )

                # epilogue
                c_t = ep.tile([128, O], F32, tag="c")
                # evict xwi (+ input bias); ready once k=7 matmuls land
                nc.vector.scalar_tensor_tensor(
                    c_t[:], xwi[:], 0.0, bib_t[:], op0=ALU.add, op1=ALU.add
                )

                stats = sp.tile([128, 24], F32, tag="stats")
                for i in range(4):
                    nc.vector.bn_stats(
                        stats[:, i * 6:(i + 1) * 6], pc[:, i * 512:(i + 1) * 512]
                    )
                mv = sp.tile([128, 2], F32, tag="mv")
                nc.vector.bn_aggr(mv[:], stats[:])
                std = sp.tile([128, 1], F32, tag="std")
                nc.scalar.activation(std[:], mv[:, 1:2], AF.Sqrt, bias=eps_t[:])
                rstd = sp.tile([128, 1], F32, tag="rstd")
                nc.vector.reciprocal(rstd[:], std[:])

                t1 = sp.tile([128, 2048], F32, tag="t1")
                # (comb - mu) * ln_w, straight out of PSUM (frees comb banks)
                nc.vector.scalar_tensor_tensor(
                    t1[:], pc[:], mv[:, 0:1], lnw_t[:],
                    op0=ALU.subtract, op1=ALU.mult,
                )
                # in-place: t1 = t1 * rstd + ln_b
                nc.vector.scalar_tensor_tensor(
                    t1[:], t1[:], rstd[:], lnb_t[:],
                    op0=ALU.mult, op1=ALU.add,
                )
                g = sp.tile([128, 2048], F32, tag="g")
                nc.scalar.activation(g[:], t1[:], AF.Sigmoid)

                a_t = ep.tile([128, O], F32, tag="a")
                # evict hwh (+ hidden bias)
                nc.vector.scalar_tensor_tensor(
                    a_t[:], hwh[:], 0.0, bhb_t[:], op0=ALU.add, op1=ALU.add
                )
                nc.vector.tensor_mul(a_t[:], a_t[:], g[:, 0:O])
                nc.vector.tensor_add(a_t[:], a_t[:], c_t[:])
                h_t = ep.tile([128, O], F32, tag="h")
                nc.scalar.activation(h_t[:], a_t[:], AF.Tanh)

                # out = hidden + fg * (state - hidden)
                nc.vector.tensor_sub(c_t[:], st_t[:], h_t[:])
                nc.vector.tensor_mul(c_t[:], c_t[:], g[:, O:2 * O])
                nc.vector.tensor_add(st_t[:], h_t[:], c_t[:])
                nc.sync.dma_start(out[bt * 128:(bt + 1) * 128, :], st_t[:])

    _split_multi_waits(nc)
    nc.finalize()
    return nc


def build_trivial():
    """1-DMA kernel used by test.py as a dispatch-overhead baseline."""
    nc = bass.Bass()
    x = nc.declare_dram_parameter("x", [128, 128], F32, isOutput=False)
    y = nc.declare_dram_parameter("y", [128, 128], F32, isOutput=True)
    with _TC(nc) as tc:
        with tc.tile_pool(name="p", bufs=1) as pool:
            t = pool.tile([128, 128], F32)
            nc.sync.dma_start(t[:], x[:])
            nc.sync.dma_start(y[:], t[:])
    _split_multi_waits(nc)
    nc.finalize()
    return nc


_NC_CACHE = {}


def _get_nc(n_bt=N_BT):
    if n_bt not in _NC_CACHE:
        _NC_CACHE[n_bt] = build_program(n_bt)
    return _NC_CACHE[n_bt]


def _prep_inputs(inpute, state, Wt, Wi, bi, Wh, bh, ln_w, ln_b):
    inpute = np.asarray(inpute, np.float32)
    state = np.asarray(state, np.float32)
    w_all = np.zeros((2048, 3072), np.float32)
    w_all[:, :2048] = np.asarray(Wt, np.float32).T
    w_all[:1024, 2048:] = np.asarray(Wi, np.float32).T
    w_all[1024:, 2048:] = np.asarray(Wh, np.float32).T
    w_all = w_all.astype(BF16)

    lnw_b = np.broadcast_to(
        np.asarray(ln_w, np.float32).reshape(1, 2048), (128, 2048)
    ).astype(BF16)
    lnb_b = np.broadcast_to(
        np.asarray(ln_b, np.float32).reshape(1, 2048), (128, 2048)
    ).astype(BF16)
    bib_b = np.broadcast_to(
        np.asarray(bi, np.float32).reshape(1, O), (128, O)
    ).astype(BF16)
    bhb_b = np.broadcast_to(
        np.asarray(bh, np.float32).reshape(1, O), (128, O)
    ).astype(BF16)

    in_maps = []
    for c in range(N_CORES):
        x_c = inpute[c * BL:(c + 1) * BL]
        h_c = state[c * BL:(c + 1) * BL]
        xt_c = np.empty((2048, BL), BF16)
        xt_c[:1024] = x_c.T.astype(BF16)
        xt_c[1024:] = h_c.T.astype(BF16)
        in_maps.append(
            {
                "xt": np.ascontiguousarray(xt_c),
                "st": np.ascontiguousarray(h_c),
                "w": w_all,
                "lnw": lnw_b,
                "lnb": lnb_b,
                "bib": bib_b,
                "bhb": bhb_b,
            }
        )
    return in_maps


def run(inputs, trace=False, **trace_kwargs):
    nc = _get_nc()
    in_maps = _prep_inputs(**inputs)
    res = run_bass_kernel_spmd(
        nc, in_maps, list(range(N_CORES)), trace=trace, **trace_kwargs
    )
    out = np.concatenate([res.results[c]["out"] for c in range(N_CORES)], axis=0)
    return out, res


def kernel(**inputs):
    out, _ = run(inputs)
    return out


# revision 13
# speedup vs baseline: 346.7899x; 346.7899x over previous
"""GRUCell4RNMT fused Trainium2 kernel, data-parallel over 8 NeuronCores.

Reference computation (per batch row b):
    comb  = concat([x, h]) @ Wt.T            # [B, 2048]
    mu, var = moments over all 2048 comb features (joint LayerNorm)
    normed  = (comb - mu) * rsqrt(var+eps) * ln_w + ln_b
    ig, fg  = sigmoid(normed).split(2)
    hidden  = tanh(x @ Wi.T + bi + ig * (h @ Wh.T + bh))
    out     = (1 - fg) * hidden + fg * h

Strategy: shard batch 16384 -> 8 x 2048. On each core the three matmuls
share one pass over the contraction dim: host packs W_all[2048, 3072] =
[Wt.T | blockdiag(Wi.T, Wh.T)] so each stationary X^T k-tile streams 6
N=512 chunks (4 comb + 2 aux) into all 8 PSUM banks. Activations are
pre-transposed on host (X^T = [x;h]^T per core) so the device does no
transposes. Matmuls run in bf16 (1 cyc/row on PE); accumulation is fp32
in PSUM; LN statistics via bn_stats/bn_aggr; epilogue on DVE/ACT.
"""

import numpy as np
import ml_dtypes

from concourse import bass, mybir, tile
from concourse.bass_utils import run_bass_kernel_spmd
from concourse.vector_clock import ScopedClock

BF16 = ml_dtypes.bfloat16
F32 = mybir.dt.float32
BF = mybir.dt.bfloat16
AF = mybir.ActivationFunctionType
ALU = mybir.AluOpType

N_CORES = 8
B = 16384
O = 1024
BL = B // N_CORES          # 2048 rows per core
N_BT = BL // 128           # 16 batch tiles per core
N_K = 16                   # contraction tiles (8 from x, 8 from h)
LN_EPS = 1e-6


class _TC(tile.TileContext):
    """TileContext whose kernel-tail drain honors the 1-wait-per-
    instruction ISA cap: extra waits move onto dedicated drains."""

    def _drain_and_barrier(self, tick_clock, wait_clock):
        drain_inst = self.nc.sync.drain()
        wait_clock.add_sem_waits(
            drain_inst.ins, ScopedClock({None: tick_clock.global_clock})
        )
        si = drain_inst.ins.sync_info
        if si is not None and si.on_wait and len(si.on_wait) > 1:
            waits = list(si.on_wait)
            SI = type(si)
            si.on_wait = [waits[0]]
            for w in waits[1:]:
                extra = self.nc.sync.drain()
                extra.ins.sync_info = SI(on_wait=[w], on_update=[])
        self.nc.all_engine_barrier()
        assert self.sems is not None
        popped = self.nc._tile_sem_poison_stack.pop()
        assert popped is self._sem_poison
        self.nc.clear_and_free_semaphores(list(self.sems.allocated().values()))
        self.nc.all_engine_barrier()


def _split_multi_waits(nc):
    """This walrus build accepts 1 sync wait per instruction (2 on
    EventSemaphore). Tile's scheduler can emit more; move the extras
    onto EventSemaphore carriers inserted just before the offender on
    the same engine (identical blocking semantics)."""
    for fn in nc.m.functions:
        for blk in fn.blocks:
            il = blk.instructions
            i = 0
            while i < len(il):
                inst = il[i]
                si = inst.sync_info
                cap = 2 if isinstance(inst, mybir.InstEventSemaphore) else 1
                if si is not None and si.on_wait and len(si.on_wait) > cap:
                    waits = list(si.on_wait)
                    SI = type(si)
                    si.on_wait = waits[:cap]
                    extra = waits[cap:]
                    pos = i
                    while extra:
                        chunk, extra = extra[:2], extra[2:]
                        ev = mybir.InstEventSemaphore(
                            name=nc.get_next_instruction_name(), ins=[], outs=[]
                        )
                        ev.engine = inst.engine
                        ev.sync_info = SI(on_wait=chunk, on_update=[])
                        nc.register_instruction(ev, overwrite=True)
                        il.insert(pos, ev)
                        pos += 1
                        i += 1
                i += 1


def build_program(n_bt=N_BT, reps=1, trace_sim=False):
    nc = bass.Bass()
    bl = n_bt * 128

    xt = nc.declare_dram_parameter("xt", [2048, bl], BF, isOutput=False)
    st = nc.declare_dram_parameter("st", [bl, O], F32, isOutput=False)
    w = nc.declare_dram_parameter("w", [2048, 3072], BF, isOutput=False)
    lnw = nc.declare_dram_parameter("lnw", [128, 2048], BF, isOutput=False)
    lnb = nc.declare_dram_parameter("lnb", [128, 2048], BF, isOutput=False)
    bib = nc.declare_dram_parameter("bib", [128, O], BF, isOutput=False)
    bhb = nc.declare_dram_parameter("bhb", [128, O], BF, isOutput=False)
    out = nc.declare_dram_parameter("out", [bl, O], F32, isOutput=True)

    # DRAM view of X^T tiled for per-batch-tile loads: [p, k, b*128+j]
    xt_r = xt[:].rearrange("(k p) n -> p k n", p=128)

    with _TC(nc, trace_sim=trace_sim) as tc:
        with (
            tc.tile_pool(name="wp", bufs=1) as wp,
            tc.tile_pool(name="cp", bufs=1) as cp,
            tc.tile_pool(name="xp", bufs=3) as xp,
            tc.tile_pool(name="sp", bufs=2) as sp,
            tc.tile_pool(name="ep", bufs=2) as ep,
            tc.tile_pool(name="pc_p", bufs=1, space="PSUM") as pc_p,
            tc.tile_pool(name="pa_p", bufs=1, space="PSUM") as pa_p,
        ):
            # Weight tiles are DMA'd lazily (2 k-chunks ahead of the
            # first batch-tile's consumption) so PE starts ~3us in
            # instead of waiting for the full 12MB weight load.
            wt = [
                wp.tile([128, 3072], BF, tag=f"w{k}", name=f"w{k}")
                for k in range(N_K)
            ]
            for k in (0, 1):
                nc.sync.dma_start(wt[k][:], w[k * 128:(k + 1) * 128, :])

            lnw_t = cp.tile([128, 2048], BF, tag="lnw")
            lnb_t = cp.tile([128, 2048], BF, tag="lnb")
            bib_t = cp.tile([128, O], BF, tag="bib")
            bhb_t = cp.tile([128, O], BF, tag="bhb")
            eps_t = cp.tile([128, 1], F32, tag="eps")

            for bt_r in range(n_bt * reps):
                bt = bt_r % n_bt
                xt_t = xp.tile([128, N_K, 128], BF, tag="xt")
                nc.sync.dma_start(
                    xt_t[:], xt_r[:, :, bt * 128:(bt + 1) * 128]
                )
                st_t = ep.tile([128, O], F32, tag="st")
                nc.sync.dma_start(st_t[:], st[bt * 128:(bt + 1) * 128, :])

                pc = pc_p.tile([128, 2048], F32, tag="pc")
                xwi = pa_p.tile([128, O], F32, tag="xwi")
                hwh = pa_p.tile([128, O], F32, tag="hwh")

                for k in range(N_K):
                    if bt_r == 0 and k + 2 < N_K:
                        kk = k + 2
                        nc.sync.dma_start(
                            wt[kk][:], w[kk * 128:(kk + 1) * 128, :]
                        )
                    lhsT = xt_t[:, k, :]
                    wk = wt[k]
                    # aux chunks first: frees those banks earliest
                    for n in (4, 5):
                        dst = xwi if k < 8 else hwh
                        nc.tensor.matmul(
                            dst[:, (n - 4) * 512:(n - 3) * 512],
                            lhsT,
                            wk[:, n * 512:(n + 1) * 512],
                            start=(k % 8 == 0),
                            stop=(k % 8 == 7),
                        )
                    for n in range(4):
                        nc.tensor.matmul(
                            pc[:, n * 512:(n + 1) * 512],
                            lhsT,
                            wk[:, n * 512:(n + 1) * 512],
                            start=(k == 0),
                            stop=(k == N_K - 1),
                        )

                if bt_r == 0:
                    nc.sync.dma_start(lnw_t[:], lnw[:])
                    nc.sync.dma_start(lnb_t[:], lnb[:])
                    nc.sync.dma_start(bib_t[:], bib[:])
                    nc.sync.dma_start(bhb_t[:], bhb[:])
                    nc.vector.memset(eps_t[:], LN_EPS)

                # epilogue
                c_t = ep.tile([128, O], F32, tag="c")
                # evict xwi (+ input bias); ready once k=7 matmuls land
                nc.vector.scalar_tensor_tensor(
                    c_t[:], xwi[:], 0.0, bib_t[:], op0=ALU.add, op1=ALU.add
                )

                stats = sp.tile([128, 24], F32, tag="stats")
                for i in range(4):
                    nc.vector.bn_stats(
                        stats[:, i * 6:(i + 1) * 6], pc[:, i * 512:(i + 1) * 512]
                    )
                mv = sp.tile([128, 2], F32, tag="mv")
                nc.vector.bn_aggr(mv[:], stats[:])
                std = sp.tile([128, 1], F32, tag="std")
                nc.scalar.activation(std[:], mv[:, 1:2], AF.Sqrt, bias=eps_t[:])
                rstd = sp.tile([128, 1], F32, tag="rstd")
                nc.vector.reciprocal(rstd[:], std[:])

                t1 = sp.tile([128, 2048], F32, tag="t1")
                # (comb - mu) * ln_w, straight out of PSUM (frees comb banks)
                nc.vector.scalar_tensor_tensor(
                    t1[:], pc[:], mv[:, 0:1], lnw_t[:],
                    op0=ALU.subtract, op1=ALU.mult,
                )
                # in-place: t1 = t1 * rstd + ln_b
                nc.vector.scalar_tensor_tensor(
                    t1[:], t1[:], rstd[:], lnb_t[:],
                    op0=ALU.mult, op1=ALU.add,
                )
                g = sp.tile([128, 2048], F32, tag="g")
                nc.scalar.activation(g[:], t1[:], AF.Sigmoid)

                a_t = ep.tile([128, O], F32, tag="a")
                # evict hwh (+ hidden bias)
                nc.vector.scalar_tensor_tensor(
                    a_t[:], hwh[:], 0.0, bhb_t[:], op0=ALU.add, op1=ALU.add
                )
                nc.vector.tensor_mul(a_t[:], a_t[:], g[:, 0:O])
                nc.vector.tensor_add(a_t[:], a_t[:], c_t[:])
                h_t = ep.tile([128, O], F32, tag="h")
                nc.scalar.activation(h_t[:], a_t[:], AF.Tanh)

                # out = hidden + fg * (state - hidden)
                nc.vector.tensor_sub(c_t[:], st_t[:], h_t[:])
                nc.vector.tensor_mul(c_t[:], c_t[:], g[:, O:2 * O])
                nc.vector.tensor_add(st_t[:], h_t[:], c_t[:])
                nc.sync.dma_start(out[bt * 128:(bt + 1) * 128, :], st_t[:])

    _split_multi_waits(nc)
    nc.finalize()
    return nc


def build_trivial():
    """1-DMA kernel used by test.py as a dispatch-overhead baseline."""
    nc = bass.Bass()
    x = nc.declare_dram_parameter("x", [128, 128], F32, isOutput=False)
    y = nc.declare_dram_parameter("y", [128, 128], F32, isOutput=True)
    with _TC(nc) as tc:
        with tc.tile_pool(name="p", bufs=1) as pool:
            t = pool.tile([128, 128], F32)
            nc.sync.dma_start(t[:], x[:])
            nc.sync.dma_start(y[:], t[:])
    _split_multi_waits(nc)
    nc.finalize()
    return nc


_NC_CACHE = {}


def _get_nc(n_bt=N_BT):
    if n_bt not in _NC_CACHE:
        _NC_CACHE[n_bt] = build_program(n_bt)
    return _NC_CACHE[n_bt]


def _prep_inputs(inpute, state, Wt, Wi, bi, Wh, bh, ln_w, ln_b):
    inpute = np.asarray(inpute, np.float32)
    state = np.asarray(state, np.float32)
    w_all = np.zeros((2048, 3072), np.float32)
    w_all[:, :2048] = np.asarray(Wt, np.float32).T
    w_all[:1024, 2048:] = np.asarray(Wi, np.float32).T
    w_all[1024:, 2048:] = np.asarray(Wh, np.float32).T
    w_all = w_all.astype(BF16)

    lnw_b = np.broadcast_to(
        np.asarray(ln_w, np.float32).reshape(1, 2048), (128, 2048)
    ).astype(BF16)
    lnb_b = np.broadcast_to(
        np.asarray(ln_b, np.float32).reshape(1, 2048), (128, 2048)
    ).astype(BF16)
    bib_b = np.broadcast_to(
        np.asarray(bi, np.float32).reshape(1, O), (128, O)
    ).astype(BF16)
    bhb_b = np.broadcast_to(
        np.asarray(bh, np.float32).reshape(1, O), (128, O)
    ).astype(BF16)

    in_maps = []
    for c in range(N_CORES):
        x_c = inpute[c * BL:(c + 1) * BL]
        h_c = state[c * BL:(c + 1) * BL]
        xt_c = np.empty((2048, BL), BF16)
        xt_c[:1024] = x_c.T.astype(BF16)
        xt_c[1024:] = h_c.T.astype(BF16)
        in_maps.append(
            {
                "xt": np.ascontiguousarray(xt_c),
                "st": np.ascontiguousarray(h_c),
                "w": w_all,
                "lnw": lnw_b,
                "lnb": lnb_b,
                "bib": bib_b,
                "bhb": bhb_b,
            }
        )
    return in_maps


def run(inputs, trace=False, **trace_kwargs):
    nc = _get_nc()
    in_maps = _prep_inputs(**inputs)
    res = run_bass_kernel_spmd(
        nc, in_maps, list(range(N_CORES)), trace=trace, **trace_kwargs
    )
    out = np.concatenate([res.results[c]["out"] for c in range(N_CORES)], axis=0)
    return out, res


def kernel(**inputs):
    out, _ = run(inputs)
    return out
